# revision 1
# baseline (speedup 1.0000x reference)
"""Multi-head attention (B=1, S=4096, D=1024, H=16, causal) on 8 Trainium2
NeuronCores.

Sharding: tensor-parallel over heads — each core owns 2 heads (128 of the
1024 projection dims). Wq/Wk/Wv are split column-wise, Wo row-wise; each
core computes a full [S, D] partial of the output projection and the
all-reduce is done on the host by summing the 8 partials (+ Wo_b once).

Per-core device kernel (all matmuls in f32r at N=512 → full PE rate):
  qT/kT/vT projections produce [c=128, S] layouts directly (contract dim D
  streams from host-pretransposed Q^T/K^T/V^T in HBM); v is PE-transposed
  per 128-block into an augmented [k, 65] layout (ones column ⇒ softmax
  denominator falls out of the attn@V matmul as PSUM row 64).
  Scores are computed transposed (scoresT[k, q] = k q^T) so softmax exp is
  the PSUM eviction (ACT, scale=1/8, additive -1e9 causal mask on the 4
  diagonal blocks only, fully-masked blocks skipped) and attn@V needs no
  transposes. Normalization (1/denom) is broadcast across partitions with a
  K=1 ones matmul, applied by DVE, and the normalized [c, q] tiles are the
  stationary operands of the final Wo matmul.
"""

import numpy as np

D = 1024
H = 16
DK = D // H  # 64
S = 4096
NCORES = 8
CD = 128          # c-dims (2 heads) per core
ST = 512          # s/q tile
NST = S // ST     # 8
KB = 128          # k block
NKB = S // KB     # 32
SLOT = 2 * (DK + 1)  # 130: v_sb cols per k-block (2 heads x (64 dims + ones))

_compiled = [None]


def _build():
    import concourse.bacc as bacc
    import concourse.mybir as mybir
    import concourse.tile as tile

    f32 = mybir.dt.float32
    f32r = mybir.dt.float32r
    EXP = mybir.ActivationFunctionType.Exp
    ADD = mybir.AluOpType.add
    MULT = mybir.AluOpType.mult

    nc = bacc.Bacc(None, target_bir_lowering=False)

    QT = nc.dram_tensor("qt", [D, S], f32r, kind="ExternalInput")
    KT = nc.dram_tensor("kt", [D, S], f32r, kind="ExternalInput")
    VT = nc.dram_tensor("vt", [D, S], f32r, kind="ExternalInput")
    WQ = nc.dram_tensor("wq", [D, CD], f32r, kind="ExternalInput")
    WK = nc.dram_tensor("wk", [D, CD], f32r, kind="ExternalInput")
    WV = nc.dram_tensor("wv", [D, CD], f32r, kind="ExternalInput")
    BQ = nc.dram_tensor("bq", [CD, 1], f32, kind="ExternalInput")
    BK = nc.dram_tensor("bk", [CD, 1], f32, kind="ExternalInput")
    BV = nc.dram_tensor("bv", [CD, 1], f32, kind="ExternalInput")
    WO0 = nc.dram_tensor("wo0", [DK, D], f32r, kind="ExternalInput")
    WO1 = nc.dram_tensor("wo1", [DK, D], f32r, kind="ExternalInput")
    MSK = nc.dram_tensor("msk", [KB, KB], f32r, kind="ExternalInput")
    ONEV = nc.dram_tensor("onev", [KB, NKB, 1], f32r, kind="ExternalInput")
    ONEP = nc.dram_tensor("onep", [1, DK], f32r, kind="ExternalInput")
    EYE = nc.dram_tensor("eye", [128, 128], f32, kind="ExternalInput")
    OUT = nc.dram_tensor("out", [S, D], f32, kind="ExternalOutput")

    def r(ap):
        return ap.bitcast(f32r)

    with tile.TileContext(nc) as tc:
        with (
            tc.tile_pool(name="const", bufs=1) as const,
            tc.tile_pool(name="qin", bufs=2) as qin_p,
            tc.tile_pool(name="kin", bufs=2) as kin_p,
            tc.tile_pool(name="vin", bufs=2) as vin_p,
            tc.tile_pool(name="vtx", bufs=2) as vtx_p,
            tc.tile_pool(name="expp", bufs=4) as exp_p,
            tc.tile_pool(name="osb", bufs=2) as osb_p,
            tc.tile_pool(name="rsb", bufs=2) as rsb_p,
            tc.tile_pool(name="oout", bufs=3) as oout_p,
            tc.tile_pool(name="wlp", bufs=6) as wl_p,
            tc.tile_pool(name="psA", bufs=2, space="PSUM") as psA,
            tc.tile_pool(name="psS", bufs=2, space="PSUM") as psS,
            tc.tile_pool(name="psO", bufs=2, space="PSUM") as psO,
        ):
            # ---- static SBUF tensors ----
            qT_sb = const.tile([CD, S], f32r, tag="qT")
            kT_sb = const.tile([CD, S], f32r, tag="kT")
            v_sb = const.tile([128, NKB, SLOT], f32r, tag="vsb")

            wq_sb = const.tile([128, D], f32r, tag="wq")
            wk_sb = const.tile([128, D], f32r, tag="wk")
            wv_sb = const.tile([128, D], f32r, tag="wv")
            woR0 = const.tile([DK, D], f32r, tag="woR0")
            woR1 = const.tile([DK, D], f32r, tag="woR1")
            mask_sb = const.tile([KB, KB], f32r, tag="mask")
            eye_sb = const.tile([128, 128], f32, tag="eye")
            bq_sb = const.tile([CD, 1], f32, tag="bq")
            bk_sb = const.tile([CD, 1], f32, tag="bk")
            bv_sb = const.tile([CD, 1], f32, tag="bv")
            onesP = const.tile([65, DK], f32r, tag="onesP")


            woL_tiles = {}

            prefetched = {}

            QTr = QT.rearrange("(g t p) s -> g p t s", g=2, p=128)
            KTr = KT.rearrange("(g t p) s -> g p t s", g=2, p=128)
            VTr = VT.rearrange("(g t p) s -> g p t s", g=2, p=128)

            def fetch(st, src_r, in_pool, g, name):
                xin = in_pool.tile([128, 4, ST], f32r, tag="xin",
                                   name=f"xin_{name}{st}_{g}")
                nc.sync.dma_start(
                    out=xin[:],
                    in_=src_r[g][:, :, st * ST : (st + 1) * ST],
                )
                return xin

            # critical consts first (first proj matmuls need these)
            for w_sb, W in ((wq_sb, WQ), (wk_sb, WK), (wv_sb, WV)):
                nc.sync.dma_start(
                    out=w_sb.rearrange("p (t c) -> p t c", c=CD),
                    in_=W.rearrange("(t p) c -> p t c", p=128),
                )
            nc.sync.dma_start(out=bq_sb[:], in_=BQ[:])
            nc.sync.dma_start(out=bk_sb[:], in_=BK[:])
            nc.sync.dma_start(out=bv_sb[:], in_=BV[:])
            for _g in range(2):
                prefetched[("q", 0, _g)] = fetch(0, QTr, qin_p, _g, "q")
                prefetched[("k", 0, _g)] = fetch(0, KTr, kin_p, _g, "k")
                prefetched[("v", 0, _g)] = fetch(0, VTr, vin_p, _g, "v")

            # bulky / later-needed consts
            nc.sync.dma_start(out=eye_sb[:], in_=EYE[:])
            nc.sync.dma_start(out=onesP[64:65, :], in_=ONEP[:])
            nc.sync.dma_start(out=v_sb[:, :, DK : DK + 1], in_=ONEV[:])
            nc.sync.dma_start(out=v_sb[:, :, SLOT - 1 : SLOT], in_=ONEV[:])
            nc.sync.dma_start(out=mask_sb[:], in_=MSK[:])
            nc.sync.dma_start(out=woR0[:], in_=WO0[:])
            nc.sync.dma_start(out=woR1[:], in_=WO1[:])

            def project(st, src_r, w_sb, b_sb, in_pool, dst_ap, name):
                """dst_ap [128, ST] = (W X + b)^T tile for s-range st."""
                ps = psA.tile([128, ST], f32, tag="pp", name=f"pp{st}")
                for g in range(2):
                    xin = prefetched.pop((name, st, g), None)
                    if xin is None:
                        xin = fetch(st, src_r, in_pool, g, name)
                    for t in range(4):
                        d = 4 * g + t
                        nc.tensor.matmul(
                            ps[:],
                            lhsT=(w_sb[:, d * CD : (d + 1) * CD]),
                            rhs=(xin[:, t, :]),
                            start=(d == 0),
                            stop=(d == 7),
                        )
                nc.vector.tensor_scalar_add(dst_ap, ps[:], b_sb[:])
                return ps

            def attn2(qt):
                nkb = 4 * qt + 4
                po = {}
                for h in (0, 1):
                    po[h] = psO.tile([65, ST], f32, tag="po",
                                     name=f"po{qt}_{h}")
                for pr in range(nkb // 2):
                    for h in (0, 1):
                        ps = psS.tile([128, 2 * ST], f32, tag="ps",
                                      name=f"ps{qt}_{h}_{pr}")
                        rels = []
                        for j in range(2):
                            kb = 2 * pr + j
                            rel = kb - 4 * qt  # >=0: diagonal block
                            rels.append(rel)
                            c0 = 128 * rel if rel > 0 else 0
                            nc.tensor.matmul(
                                ps[:, j * ST + c0 : (j + 1) * ST],
                                lhsT=(kT_sb[64 * h : 64 * h + 64,
                                             kb * KB : (kb + 1) * KB]),
                                rhs=(qT_sb[64 * h : 64 * h + 64,
                                            qt * ST + c0 : (qt + 1) * ST]),
                                start=True,
                                stop=True,
                            )
                        ex = exp_p.tile([128, 2 * ST], f32r, tag="ex",
                                        name=f"ex{qt}_{h}_{pr}")
                        if rels[0] >= 2:  # steep diagonal pair: narrow exps
                            for j in range(2):
                                c0 = 128 * rels[j]
                                nc.scalar.activation(
                                    ex[:, j * ST + c0 : (j + 1) * ST],
                                    ps[:, j * ST + c0 : (j + 1) * ST],
                                    EXP, scale=0.125,
                                )
                        else:
                            nc.scalar.activation(ex[:], ps[:], EXP,
                                                 scale=0.125)
                        for j in range(2):
                            rel = rels[j]
                            if rel >= 0:  # zero the partial 128-band
                                b0 = j * ST + 128 * rel
                                nc.vector.tensor_tensor(
                                    out=ex[:, b0 : b0 + 128],
                                    in0=ex[:, b0 : b0 + 128],
                                    in1=mask_sb[:],
                                    op=MULT,
                                )
                        for j in range(2):
                            kb = 2 * pr + j
                            rel = kb - 4 * qt
                            c0 = 128 * rel if rel > 0 else 0
                            nc.tensor.matmul(
                                po[h][:, c0:ST],
                                lhsT=(v_sb[:, kb, h * 65 : h * 65 + 65]),
                                rhs=(ex[:, j * ST + c0 : (j + 1) * ST]),
                                start=(pr == 0 and j == 0),
                                stop=(pr == nkb // 2 - 1 and j == 1),
                            )
                for h in (0, 1):
                    o_sb = osb_p.tile([65, ST], f32r, tag="o",
                                      name=f"o{qt}_{h}")
                    nc.vector.tensor_copy(o_sb[:], po[h][:])
                    pb = psO.tile([DK, ST], f32, tag="po", name=f"pb{qt}_{h}")
                    nc.tensor.matmul(
                        pb[:], lhsT=onesP[64:65, :], rhs=o_sb[64:65, :],
                        start=True, stop=True,
                    )
                    r_sb = rsb_p.tile([DK, ST], f32, tag="r",
                                      name=f"r{qt}_{h}")
                    nc.vector.reciprocal_approx_fast(out=r_sb[:], in_=pb[:])
                    woL = wl_p.tile([DK, ST], f32r, tag="wl",
                                    name=f"wl{qt}_{h}")
                    nc.vector.tensor_tensor(
                        out=woL[:], in0=o_sb[0:64, :], in1=r_sb[:], op=MULT,
                    )
                    woL_tiles[(qt, h)] = woL

            def wo(qt):
                wl0 = woL_tiles.pop((qt, 0))
                wl1 = woL_tiles.pop((qt, 1))
                for qb in range(4):
                    q0 = qt * ST + qb * 128
                    for nt in range(2):
                        pw = psA.tile([128, ST], f32, tag="pp",
                                      name=f"pw{qt}_{qb}_{nt}")
                        nc.tensor.matmul(
                            pw[:],
                            lhsT=(wl0[:, qb * 128 : (qb + 1) * 128]),
                            rhs=(woR0[:, nt * ST : (nt + 1) * ST]),
                            start=True, stop=False,
                        )
                        nc.tensor.matmul(
                            pw[:],
                            lhsT=(wl1[:, qb * 128 : (qb + 1) * 128]),
                            rhs=(woR1[:, nt * ST : (nt + 1) * ST]),
                            start=False, stop=True,
                        )
                        ob = oout_p.tile([128, ST], f32, tag="ob",
                                         name=f"ob{qt}_{qb}_{nt}")
                        nc.vector.tensor_copy(ob[:], pw[:])
                        nc.sync.dma_start(
                            out=OUT[q0 : q0 + 128, nt * ST : (nt + 1) * ST],
                            in_=ob[:],
                        )

            for st in range(NST):
                project(st, QTr, wq_sb, bq_sb, qin_p,
                        qT_sb[:, st * ST : (st + 1) * ST], "q")
                project(st, KTr, wk_sb, bk_sb, kin_p,
                        kT_sb[:, st * ST : (st + 1) * ST], "k")
                vtx = vtx_p.tile([128, ST], f32, tag="vtx", name=f"vtx{st}")
                project(st, VTr, wv_sb, bv_sb, vin_p, vtx[:], "v")
                # transpose vT [c, s] blocks into v_sb [s, c] aug slots
                for qb in range(4):
                    kb = 4 * st + qb
                    tp = psA.tile([128, 128], f32, tag="pp", name=f"pt{kb}")
                    nc.tensor.transpose(
                        tp[:], vtx[:, qb * 128 : (qb + 1) * 128], eye_sb[:]
                    )
                    nc.vector.tensor_copy(v_sb[:, kb, 0:DK], tp[:, 0:DK])
                    nc.vector.tensor_copy(
                        v_sb[:, kb, DK + 1 : SLOT - 1], tp[:, DK:CD]
                    )
                attn2(st)
                if st > 1:
                    wo(st - 2)
            wo(NST - 2)
            wo(NST - 1)

    nc.compile()
    return nc


def _prep_inputs(Q, K, V, Wq_w, Wq_b, Wk_w, Wk_b, Wv_w, Wv_b, Wo_w, Wo_b):
    f = np.float32
    QT = np.ascontiguousarray(Q[0].T, dtype=f)
    KT = np.ascontiguousarray(K[0].T, dtype=f)
    VT = np.ascontiguousarray(V[0].T, dtype=f)
    # diagonal-block additive causal masks: m[p, r*ST + f] = 0 if 128r+p<=f
    p = np.arange(KB)[:, None]
    fidx = np.arange(KB)[None, :]
    msk = np.where(p <= fidx, 1.0, 0.0).astype(f)
    eye = np.eye(128, dtype=f)
    WoT = np.ascontiguousarray(Wo_w.T, dtype=f)  # [in, out]

    in_maps = []
    for c in range(NCORES):
        c0 = CD * c
        in_maps.append({
            "qt": QT, "kt": KT, "vt": VT,
            "wq": np.ascontiguousarray(Wq_w[c0 : c0 + CD, :].T, dtype=f),
            "wk": np.ascontiguousarray(Wk_w[c0 : c0 + CD, :].T, dtype=f),
            "wv": np.ascontiguousarray(Wv_w[c0 : c0 + CD, :].T, dtype=f),
            "bq": np.ascontiguousarray(Wq_b[c0 : c0 + CD, None], dtype=f),
            "bk": np.ascontiguousarray(Wk_b[c0 : c0 + CD, None], dtype=f),
            "bv": np.ascontiguousarray(Wv_b[c0 : c0 + CD, None], dtype=f),
            "wo0": np.ascontiguousarray(WoT[c0 : c0 + DK, :], dtype=f),
            "wo1": np.ascontiguousarray(WoT[c0 + DK : c0 + CD, :], dtype=f),
            "msk": msk, "eye": eye,
            "onev": np.ones((KB, NKB, 1), f),
            "onep": np.ones((1, DK), f),
        })
    return in_maps


def _numpy_fallback(Q, K, V, Wq_w, Wq_b, Wk_w, Wk_b, Wv_w, Wv_b, Wo_w, Wo_b,
                    mask):
    q = (Q @ Wq_w.T + Wq_b).reshape(1, S, H, DK).transpose(0, 2, 1, 3)
    k = (K @ Wk_w.T + Wk_b).reshape(1, S, H, DK).transpose(0, 2, 1, 3)
    v = (V @ Wv_w.T + Wv_b).reshape(1, S, H, DK).transpose(0, 2, 1, 3)
    scores = np.einsum("bhqd,bhkd->bhqk", q, k) / np.sqrt(DK).astype(np.float32)
    scores = np.where(mask == 0, np.float32(-1e9), scores)
    scores -= scores.max(axis=-1, keepdims=True)
    e = np.exp(scores)
    attn = e / e.sum(axis=-1, keepdims=True)
    out = np.einsum("bhqk,bhkd->bhqd", attn, v)
    out = out.transpose(0, 2, 1, 3).reshape(1, S, D)
    return (out @ Wo_w.T + Wo_b).astype(np.float32)


def kernel(Q, K, V, Wq_w, Wq_b, Wk_w, Wk_b, Wv_w, Wv_b, Wo_w, Wo_b, mask,
           **run_kwargs):
    Q = np.asarray(Q); K = np.asarray(K); V = np.asarray(V)
    Wq_w = np.asarray(Wq_w); Wq_b = np.asarray(Wq_b)
    Wk_w = np.asarray(Wk_w); Wk_b = np.asarray(Wk_b)
    Wv_w = np.asarray(Wv_w); Wv_b = np.asarray(Wv_b)
    Wo_w = np.asarray(Wo_w); Wo_b = np.asarray(Wo_b)
    mask = np.asarray(mask)

    causal = np.array_equal(
        mask.reshape(S, S), np.tril(np.ones((S, S), mask.dtype))
    )
    if not causal:
        return _numpy_fallback(Q, K, V, Wq_w, Wq_b, Wk_w, Wk_b, Wv_w, Wv_b,
                               Wo_w, Wo_b, mask)

    from concourse.bass_utils import run_bass_kernel_spmd

    if _compiled[0] is None:
        _compiled[0] = _build()
    nc = _compiled[0]

    in_maps = _prep_inputs(Q, K, V, Wq_w, Wq_b, Wk_w, Wk_b, Wv_w, Wv_b,
                           Wo_w, Wo_b)
    res = run_bass_kernel_spmd(nc, in_maps, list(range(NCORES)), **run_kwargs)
    out = np.zeros((S, D), np.float32)
    for cres in res.results:
        out += cres["out"]
    out += Wo_b.astype(np.float32)
    if run_kwargs:
        kernel.last_result = res
    return out.reshape(1, S, D)



# revision 11
# speedup vs baseline: 1.1885x; 1.1885x over previous
"""Multi-head attention (B=1, S=4096, D=1024, H=16, causal) on 8 Trainium2
NeuronCores.

Sharding: tensor-parallel over heads — each core owns 2 heads (128 of the
1024 projection dims). Wq/Wk/Wv are split column-wise, Wo row-wise; each
core computes a full [S, D] partial of the output projection (bf16) and the
all-reduce is done on the host by summing the 8 partials (+ Wo_b once).

All matmul operands are bf16 (f32 PSUM accumulation): same 1 cycle/row PE
rate as f32r but FWL-eligible weight loads, half the DMA/SBUF traffic, and
2x DVE modes where applicable.

Per-core device kernel:
  qT/kT projections produce [c=128, S] bf16 directly (contract D streams
  from host-pretransposed inputs); the two heads live on partition halves
  0-63 / 64-127 so the per-head score matmuls (contract 64) auto-derive
  PE row tiles (0,0)/(64,0) and run concurrently in the array.
  v is projected directly into [s, c] layout (x-subtile stationary) and
  bias-added into an augmented [s, 65]-per-head slot (ones column => softmax
  denominator falls out of the attn@V matmul as PSUM row 64).
  Scores are computed transposed (scoresT[k, q]) so softmax exp is the PSUM
  eviction (ACT, scale=1/8, bf16 out); the partial diagonal 128-bands are
  zeroed by a Pool-engine mask multiply; fully-masked blocks are skipped.
  Normalization (1/denom) is broadcast across partitions with a K=1 ones
  matmul; the normalized bf16 [c, q] tiles for both heads land in one
  [128, q] tile so the final Wo projection is a single K=128 matmul per
  output block, interleaved into the next attention tile's PE stream.
"""

import numpy as np
import ml_dtypes

D = 1024
H = 16
DK = D // H  # 64
S = 4096
NCORES = 8
CD = 128          # c-dims (2 heads) per core
ST = 512          # s/q tile
NST = S // ST     # 8
KB = 128          # k block
NKB = S // KB     # 32
SLOT = 65         # v_sb cols per head per k-block (64 dims + ones)

_compiled = [None]


def _build():
    import concourse.bacc as bacc
    import concourse.mybir as mybir
    import concourse.tile as tile

    f32 = mybir.dt.float32
    f32r = mybir.dt.float32r
    bf16 = mybir.dt.bfloat16
    EXP = mybir.ActivationFunctionType.Exp
    MULT = mybir.AluOpType.mult
    ADD = mybir.AluOpType.add

    nc = bacc.Bacc(None, target_bir_lowering=False)

    XQ = nc.dram_tensor("xq", [128, 8, S], bf16, kind="ExternalInput")
    XK = nc.dram_tensor("xk", [128, 8, S], bf16, kind="ExternalInput")
    XV = nc.dram_tensor("xv", [128, 8, S], bf16, kind="ExternalInput")
    WQ = nc.dram_tensor("wq", [128, 8, CD], bf16, kind="ExternalInput")
    WK = nc.dram_tensor("wk", [128, 8, CD], bf16, kind="ExternalInput")
    WV = nc.dram_tensor("wv", [128, 8, CD], bf16, kind="ExternalInput")
    BQ = nc.dram_tensor("bq", [CD, 1], f32, kind="ExternalInput")
    BK = nc.dram_tensor("bk", [CD, 1], f32, kind="ExternalInput")
    BVB = nc.dram_tensor("bvb", [128, 512], bf16, kind="ExternalInput")
    WOR = nc.dram_tensor("wor", [CD, D], bf16, kind="ExternalInput")
    MSK = nc.dram_tensor("msk", [KB, KB], bf16, kind="ExternalInput")
    ONEP = nc.dram_tensor("onep", [1, DK], f32r, kind="ExternalInput")
    OUT = nc.dram_tensor("out", [S, D], bf16, kind="ExternalOutput")

    with tile.TileContext(nc) as tc:
        with (
            tc.tile_pool(name="const", bufs=1) as const,
            tc.tile_pool(name="qin", bufs=2) as qin_p,
            tc.tile_pool(name="kin", bufs=2) as kin_p,
            tc.tile_pool(name="vin", bufs=2) as vin_p,
            tc.tile_pool(name="expp", bufs=6) as exp_p,
            tc.tile_pool(name="denp", bufs=4) as den_p,
            tc.tile_pool(name="rsb", bufs=4) as rsb_p,
            tc.tile_pool(name="wlp", bufs=3) as wl_p,
            tc.tile_pool(name="oout", bufs=4) as oout_p,
            tc.tile_pool(name="psA", bufs=2, space="PSUM") as psA,
            tc.tile_pool(name="psS", bufs=2, space="PSUM") as psS,
            tc.tile_pool(name="psO", bufs=2, space="PSUM") as psO,
        ):
            # ---- static SBUF tensors ----
            qT_sb = const.tile([CD, S], bf16, tag="qT")
            kT_sb = const.tile([CD, S], bf16, tag="kT")
            v_sb = const.tile([128, NKB, 2 * SLOT], bf16, tag="vsb")

            wq_sb = const.tile([128, 8, CD], bf16, tag="wq")
            wk_sb = const.tile([128, 8, CD], bf16, tag="wk")
            wv_sb = const.tile([128, 8, CD], bf16, tag="wv")
            woR = const.tile([CD, D], bf16, tag="woR")
            mask_sb = const.tile([KB, KB], bf16, tag="mask")
            bq_sb = const.tile([CD, 1], f32, tag="bq")
            bk_sb = const.tile([CD, 1], f32, tag="bk")
            bvb_sb = const.tile([128, 512], bf16, tag="bvb")
            onesP = const.tile([1, DK], f32r, tag="onesP")

            woL_tiles = {}
            prefetched = {}

            def fetch(st, src, in_pool, name):
                xin = in_pool.tile([128, 8, ST], bf16, tag="xin",
                                   name=f"xin_{name}{st}")
                nc.sync.dma_start(
                    out=xin[:], in_=src[:, :, st * ST : (st + 1) * ST],
                )
                return xin

            # critical consts first (first proj matmuls need these)
            nc.sync.dma_start(out=wq_sb[:], in_=WQ[:])
            nc.sync.dma_start(out=wk_sb[:], in_=WK[:])
            nc.sync.dma_start(out=wv_sb[:], in_=WV[:])
            nc.sync.dma_start(out=bq_sb[:], in_=BQ[:])
            nc.sync.dma_start(out=bk_sb[:], in_=BK[:])
            nc.sync.dma_start(out=bvb_sb[:], in_=BVB[:])
            prefetched[("q", 0)] = fetch(0, XQ, qin_p, "q")
            prefetched[("k", 0)] = fetch(0, XK, kin_p, "k")
            prefetched[("v", 0)] = fetch(0, XV, vin_p, "v")

            # bulky / later-needed consts
            nc.sync.dma_start(out=mask_sb[:], in_=MSK[:])
            nc.sync.dma_start(out=onesP[:], in_=ONEP[:])
            nc.sync.dma_start(out=woR[:], in_=WOR[:])

            # ones columns of the augmented v slots (col 64 per head slot)
            nc.gpsimd.memset(v_sb[:, :, SLOT - 1 : SLOT], 1.0)
            nc.gpsimd.memset(v_sb[:, :, 2 * SLOT - 1 : 2 * SLOT], 1.0)

            def get_in(st, name, src, in_pool):
                xin = prefetched.pop((name, st), None)
                if xin is None:
                    xin = fetch(st, src, in_pool, name)
                return xin

            v4 = v_sb.rearrange("p n (h c) -> p n h c", h=2)
            bvb4 = bvb_sb.rearrange("p (k h c) -> p k h c", k=4, h=2)

            def project_qk(st, xin, w_sb, b_sb, dst_ap, nm):
                """dst_ap [128, ST] bf16 = (W X + b)^T tile for s-range st."""
                ps = psA.tile([128, ST], f32, tag="pp", name=f"pp{nm}{st}")
                for t in range(8):
                    nc.tensor.matmul(
                        ps[:],
                        lhsT=w_sb[:, t, :],
                        rhs=xin[:, t, :],
                        start=(t == 0),
                        stop=(t == 7),
                    )
                nc.vector.tensor_scalar_add(dst_ap, ps[:], b_sb[:])

            def project_v(st, xin):
                """v_sb[:, 4st:4st+4, slots] = (X_block^T Wv + bv) in [s, c]."""
                pv = psA.tile([128, 4, 128], f32, tag="pp", name=f"pv{st}")
                for qb in range(4):
                    for t in range(8):
                        nc.tensor.matmul(
                            pv[:, qb, :],
                            lhsT=xin[:, t, qb * 128 : (qb + 1) * 128],
                            rhs=wv_sb[:, t, :],
                            start=(t == 0),
                            stop=(t == 7),
                        )
                # bias-add + pack into augmented slots (skip ones columns)
                # (DVE: GPSIMD cannot read PSUM)
                nc.vector.tensor_tensor(
                    out=v4[:, 4 * st : 4 * st + 4, :, 0:DK],
                    in0=pv.rearrange("p k (h c) -> p k h c", h=2),
                    in1=bvb4[:],
                    op=ADD,
                )

            def attn(qt, filler):
                npr = 2 * qt + 2
                po = {}
                for h in (0, 1):
                    po[h] = psO.tile([65, ST], f32, tag="po",
                                     name=f"po{qt}_{h}")

                def attnv(pr, ex):
                    # attn @ V (+ones col => denominators in PSUM row 64)
                    for h in (0, 1):
                        for j in range(2):
                            kb = 2 * pr + j
                            rel = kb - 4 * qt
                            c0 = 128 * rel if rel > 0 else 0
                            nc.tensor.matmul(
                                po[h][:, c0:ST],
                                lhsT=v_sb[:, kb, h * SLOT : (h + 1) * SLOT],
                                rhs=ex[h][:, j * ST + c0 : (j + 1) * ST],
                                start=(pr == 0 and j == 0),
                                stop=(pr == npr - 1 and j == 1),
                            )

                prev = None  # (pr, ex) whose attn@V is still pending
                for pr in range(npr):
                    rels = [2 * pr + j - 4 * qt for j in (0, 1)]
                    ps = {}
                    for h in (0, 1):
                        ps[h] = psS.tile([128, 2 * ST], f32, tag="ps",
                                         name=f"ps{qt}_{h}_{pr}")
                    # scores: head-interleaved so the two K=64 matmuls run
                    # in different PE row-groups concurrently
                    for j in range(2):
                        kb = 2 * pr + j
                        rel = rels[j]
                        c0 = 128 * rel if rel > 0 else 0
                        for h in (0, 1):
                            nc.tensor.matmul(
                                ps[h][:, j * ST + c0 : (j + 1) * ST],
                                lhsT=kT_sb[64 * h : 64 * h + 64,
                                           kb * KB : (kb + 1) * KB],
                                rhs=qT_sb[64 * h : 64 * h + 64,
                                          qt * ST + c0 : (qt + 1) * ST],
                                start=True,
                                stop=True,
                            )
                    ex = {}
                    for h in (0, 1):
                        ex[h] = exp_p.tile([128, 2 * ST], bf16, tag="ex",
                                           name=f"ex{qt}_{h}_{pr}")
                        if rels[0] >= 2:  # steep diagonal pair: narrow exps
                            for j in range(2):
                                c0 = 128 * rels[j]
                                nc.scalar.activation(
                                    ex[h][:, j * ST + c0 : (j + 1) * ST],
                                    ps[h][:, j * ST + c0 : (j + 1) * ST],
                                    EXP, scale=0.125,
                                )
                        else:
                            nc.scalar.activation(ex[h][:], ps[h][:], EXP,
                                                 scale=0.125)
                    # zero the partial diagonal 128-bands (Pool engine)
                    for h in (0, 1):
                        for j in range(2):
                            rel = rels[j]
                            if rel >= 0:
                                b0 = j * ST + 128 * rel
                                nc.gpsimd.tensor_tensor(
                                    out=ex[h][:, b0 : b0 + 128],
                                    in0=ex[h][:, b0 : b0 + 128],
                                    in1=mask_sb[:],
                                    op=MULT,
                                )
                    # attn@V lags one pair so PE never stalls on this exp
                    if prev is not None:
                        attnv(*prev)
                    prev = (pr, ex)
                    filler(2)  # interleave pending Wo output blocks
                attnv(*prev)
                # normalize: woL[h*64:(h+1)*64, :] = po[h][0:64] / denom
                woL = wl_p.tile([128, ST], bf16, tag="wl", name=f"wl{qt}")
                for h in (0, 1):
                    den = den_p.tile([1, ST], f32r, tag="den",
                                     name=f"den{qt}_{h}")
                    nc.vector.tensor_copy(den[:], po[h][64:65, :])
                    pb = psA.tile([DK, ST], f32, tag="pp", name=f"pb{qt}_{h}")
                    nc.tensor.matmul(
                        pb[:], lhsT=onesP[:], rhs=den[:],
                        start=True, stop=True,
                    )
                    r_sb = rsb_p.tile([DK, ST], f32, tag="r",
                                      name=f"r{qt}_{h}")
                    nc.vector.reciprocal_approx_fast(out=r_sb[:], in_=pb[:])
                    nc.vector.tensor_tensor(
                        out=woL[64 * h : 64 * h + 64, :],
                        in0=po[h][0:64, :], in1=r_sb[:], op=MULT,
                    )
                woL_tiles[qt] = woL

            def wo_units(qt):
                """8 closures, each one output block of the Wo projection."""
                wl = woL_tiles.pop(qt)

                def unit(qb, nt):
                    def run():
                        q0 = qt * ST + qb * 128
                        pw = psA.tile([128, ST], f32, tag="pp",
                                      name=f"pw{qt}_{qb}_{nt}")
                        nc.tensor.matmul(
                            pw[:],
                            lhsT=wl[:, qb * 128 : (qb + 1) * 128],
                            rhs=woR[:, nt * ST : (nt + 1) * ST],
                            start=True, stop=True,
                        )
                        ob = oout_p.tile([128, ST], bf16, tag="ob",
                                         name=f"ob{qt}_{qb}_{nt}")
                        nc.vector.tensor_copy(ob[:], pw[:])
                        nc.sync.dma_start(
                            out=OUT[q0 : q0 + 128, nt * ST : (nt + 1) * ST],
                            in_=ob[:],
                        )
                    return run

                return [unit(qb, nt) for qb in range(4) for nt in range(2)]

            pending = []

            def filler(n):
                for _ in range(min(n, len(pending))):
                    pending.pop(0)()

            for st in range(NST):
                xq = get_in(st, "q", XQ, qin_p)
                xk = get_in(st, "k", XK, kin_p)
                xv = get_in(st, "v", XV, vin_p)
                project_qk(st, xq, wq_sb, bq_sb,
                           qT_sb[:, st * ST : (st + 1) * ST], "q")
                project_qk(st, xk, wk_sb, bk_sb,
                           kT_sb[:, st * ST : (st + 1) * ST], "k")
                project_v(st, xv)
                if st + 1 < NST:
                    prefetched[("q", st + 1)] = fetch(st + 1, XQ, qin_p, "q")
                    prefetched[("k", st + 1)] = fetch(st + 1, XK, kin_p, "k")
                    prefetched[("v", st + 1)] = fetch(st + 1, XV, vin_p, "v")
                if st >= 1:
                    # wo for qt=st-1: drained by filler inside attn(st)
                    pending.extend(wo_units(st - 1))
                attn(st, filler)
                assert not pending
            # drain the tail
            pending.extend(wo_units(NST - 1))
            filler(len(pending))

    nc.compile()
    return nc


def _prep_inputs(Q, K, V, Wq_w, Wq_b, Wk_w, Wk_b, Wv_w, Wv_b, Wo_w, Wo_b):
    bf = ml_dtypes.bfloat16
    f = np.float32

    def xprep(X):
        # [S, D] -> [128, 8, S]: x[p, t, s] = X[s, t*128+p]
        return np.ascontiguousarray(
            X[0].T.reshape(8, 128, S).transpose(1, 0, 2).astype(bf)
        )

    def wprep(Wslice):
        # Wslice [CD, D] -> [128, 8, CD]: w[p, t, c] = Wslice[c, t*128+p]
        return np.ascontiguousarray(
            Wslice.T.reshape(8, 128, CD).transpose(1, 0, 2).astype(bf)
        )

    XQp, XKp, XVp = xprep(Q), xprep(K), xprep(V)
    p = np.arange(KB)[:, None]
    fidx = np.arange(KB)[None, :]
    msk = np.where(p <= fidx, 1.0, 0.0).astype(bf)
    WoT = np.ascontiguousarray(Wo_w.T, dtype=f)  # [in, out]

    in_maps = []
    for c in range(NCORES):
        c0 = CD * c
        in_maps.append({
            "xq": XQp, "xk": XKp, "xv": XVp,
            "wq": wprep(Wq_w[c0 : c0 + CD, :]),
            "wk": wprep(Wk_w[c0 : c0 + CD, :]),
            "wv": wprep(Wv_w[c0 : c0 + CD, :]),
            "bq": np.ascontiguousarray(Wq_b[c0 : c0 + CD, None], dtype=f),
            "bk": np.ascontiguousarray(Wk_b[c0 : c0 + CD, None], dtype=f),
            "bvb": np.ascontiguousarray(
                np.broadcast_to(np.tile(Wv_b[c0 : c0 + CD], 4), (128, 512))
            ).astype(bf),
            "wor": np.ascontiguousarray(WoT[c0 : c0 + CD, :], dtype=bf),
            "msk": msk,
            "onep": np.ones((1, DK), f),
        })
    return in_maps


def _numpy_fallback(Q, K, V, Wq_w, Wq_b, Wk_w, Wk_b, Wv_w, Wv_b, Wo_w, Wo_b,
                    mask):
    q = (Q @ Wq_w.T + Wq_b).reshape(1, S, H, DK).transpose(0, 2, 1, 3)
    k = (K @ Wk_w.T + Wk_b).reshape(1, S, H, DK).transpose(0, 2, 1, 3)
    v = (V @ Wv_w.T + Wv_b).reshape(1, S, H, DK).transpose(0, 2, 1, 3)
    scores = np.einsum("bhqd,bhkd->bhqk", q, k) / np.sqrt(DK).astype(np.float32)
    scores = np.where(mask == 0, np.float32(-1e9), scores)
    scores -= scores.max(axis=-1, keepdims=True)
    e = np.exp(scores)
    attn = e / e.sum(axis=-1, keepdims=True)
    out = np.einsum("bhqk,bhkd->bhqd", attn, v)
    out = out.transpose(0, 2, 1, 3).reshape(1, S, D)
    return (out @ Wo_w.T + Wo_b).astype(np.float32)


def kernel(Q, K, V, Wq_w, Wq_b, Wk_w, Wk_b, Wv_w, Wv_b, Wo_w, Wo_b, mask,
           **run_kwargs):
    Q = np.asarray(Q); K = np.asarray(K); V = np.asarray(V)
    Wq_w = np.asarray(Wq_w); Wq_b = np.asarray(Wq_b)
    Wk_w = np.asarray(Wk_w); Wk_b = np.asarray(Wk_b)
    Wv_w = np.asarray(Wv_w); Wv_b = np.asarray(Wv_b)
    Wo_w = np.asarray(Wo_w); Wo_b = np.asarray(Wo_b)
    mask = np.asarray(mask)

    causal = np.array_equal(
        mask.reshape(S, S), np.tril(np.ones((S, S), mask.dtype))
    )
    if not causal:
        return _numpy_fallback(Q, K, V, Wq_w, Wq_b, Wk_w, Wk_b, Wv_w, Wv_b,
                               Wo_w, Wo_b, mask)

    from concourse.bass_utils import run_bass_kernel_spmd

    if _compiled[0] is None:
        _compiled[0] = _build()
    nc = _compiled[0]

    in_maps = _prep_inputs(Q, K, V, Wq_w, Wq_b, Wk_w, Wk_b, Wv_w, Wv_b,
                           Wo_w, Wo_b)
    res = run_bass_kernel_spmd(nc, in_maps, list(range(NCORES)), **run_kwargs)
    out = np.zeros((S, D), np.float32)
    for cres in res.results:
        out += np.asarray(cres["out"], dtype=np.float32)
    out += Wo_b.astype(np.float32)
    if run_kwargs:
        kernel.last_result = res
    return out.reshape(1, S, D).astype(np.float32)


# revision 20
# speedup vs baseline: 1.2761x; 1.0736x over previous
"""Multi-head attention (B=1, S=4096, D=1024, H=16, causal) on 8 Trainium2
NeuronCores.

Sharding: tensor-parallel over heads — each core owns 2 heads (128 of the
1024 projection dims). Wq/Wk/Wv are split column-wise, Wo row-wise; each
core computes a full [S, D] partial of the output projection (bf16) and the
all-reduce is done on the host by summing the 8 partials (+ Wo_b once).

All matmul operands are bf16 (f32 PSUM accumulation): same 1 cycle/row PE
rate as f32r but FWL-eligible weight loads, half the DMA/SBUF traffic, and
2x DVE modes where applicable.

Per-core device kernel:
  qT/kT projections produce [c=128, S] bf16 directly (contract D streams
  from host-pretransposed inputs); the two heads live on partition halves
  0-63 / 64-127 so the per-head score matmuls (contract 64) auto-derive
  PE row tiles (0,0)/(64,0) and run concurrently in the array.
  v is projected directly into [s, c] layout (x-subtile stationary) and
  bias-added into an augmented [s, 65]-per-head slot (ones column => softmax
  denominator falls out of the attn@V matmul as PSUM row 64).
  Scores are computed transposed (scoresT[k, q]) so softmax exp is the PSUM
  eviction (ACT, scale=1/8, bf16 out); the partial diagonal 128-bands are
  zeroed by a Pool-engine mask multiply; fully-masked blocks are skipped.
  Normalization (1/denom) is broadcast across partitions with a K=1 ones
  matmul; the normalized bf16 [c, q] tiles for both heads land in one
  [128, q] tile so the final Wo projection is a single K=128 matmul per
  output block, interleaved into the next attention tile's PE stream.
"""

import numpy as np
import ml_dtypes

D = 1024
H = 16
DK = D // H  # 64
S = 4096
NCORES = 8
CD = 128          # c-dims (2 heads) per core
ST = 512          # s/q tile
NST = S // ST     # 8
KB = 128          # k block
NKB = S // KB     # 32
SLOT = 65         # v_sb cols per head per k-block (64 dims + ones)

_compiled = [None]


def _build():
    import concourse.bacc as bacc
    import concourse.mybir as mybir
    import concourse.tile as tile

    f32 = mybir.dt.float32
    f32r = mybir.dt.float32r
    bf16 = mybir.dt.bfloat16
    EXP = mybir.ActivationFunctionType.Exp
    MULT = mybir.AluOpType.mult
    ADD = mybir.AluOpType.add

    nc = bacc.Bacc(None, target_bir_lowering=False)

    XQ = nc.dram_tensor("xq", [128, 8, S], bf16, kind="ExternalInput")
    XK = nc.dram_tensor("xk", [128, 8, S], bf16, kind="ExternalInput")
    XV = nc.dram_tensor("xv", [128, 8, S], bf16, kind="ExternalInput")
    WQ = nc.dram_tensor("wq", [128, 8, CD], bf16, kind="ExternalInput")
    WK = nc.dram_tensor("wk", [128, 8, CD], bf16, kind="ExternalInput")
    WV = nc.dram_tensor("wv", [128, 8, CD], bf16, kind="ExternalInput")
    BQ = nc.dram_tensor("bq", [CD, 1], f32, kind="ExternalInput")
    BK = nc.dram_tensor("bk", [CD, 1], f32, kind="ExternalInput")
    BVB = nc.dram_tensor("bvb", [128, 512], bf16, kind="ExternalInput")
    WOR = nc.dram_tensor("wor", [CD, D], bf16, kind="ExternalInput")
    MSK = nc.dram_tensor("msk", [KB, KB], bf16, kind="ExternalInput")
    ONEP = nc.dram_tensor("onep", [1, DK], f32r, kind="ExternalInput")
    OUT = nc.dram_tensor("out", [S, D], bf16, kind="ExternalOutput")

    with tile.TileContext(nc) as tc:
        with (
            tc.tile_pool(name="const", bufs=1) as const,
            tc.tile_pool(name="qin", bufs=2) as qin_p,
            tc.tile_pool(name="kin", bufs=2) as kin_p,
            tc.tile_pool(name="vin", bufs=2) as vin_p,
            tc.tile_pool(name="expp", bufs=6) as exp_p,
            tc.tile_pool(name="denp", bufs=4) as den_p,
            tc.tile_pool(name="rsb", bufs=4) as rsb_p,
            tc.tile_pool(name="wlp", bufs=3) as wl_p,
            tc.tile_pool(name="oout", bufs=4) as oout_p,
            tc.tile_pool(name="psA", bufs=2, space="PSUM") as psA,
            tc.tile_pool(name="psS", bufs=2, space="PSUM") as psS,
            tc.tile_pool(name="psO", bufs=2, space="PSUM") as psO,
        ):
            # ---- static SBUF tensors ----
            qT_sb = const.tile([CD, S], bf16, tag="qT")
            kT_sb = const.tile([CD, S], bf16, tag="kT")
            v_sb = const.tile([128, NKB, 2 * SLOT], bf16, tag="vsb")

            wq_sb = const.tile([128, 8, CD], bf16, tag="wq")
            wk_sb = const.tile([128, 8, CD], bf16, tag="wk")
            wv_sb = const.tile([128, 8, CD], bf16, tag="wv")
            woR = const.tile([CD, D], bf16, tag="woR")
            mask_sb = const.tile([KB, KB], bf16, tag="mask")
            bq_sb = const.tile([CD, 1], f32, tag="bq")
            bk_sb = const.tile([CD, 1], f32, tag="bk")
            bvb_sb = const.tile([128, 512], bf16, tag="bvb")
            onesP = const.tile([1, DK], f32r, tag="onesP")

            woL_tiles = {}
            prefetched = {}

            def fetch(st, src, in_pool, name):
                xin = in_pool.tile([128, 8, ST], bf16, tag="xin",
                                   name=f"xin_{name}{st}")
                nc.sync.dma_start(
                    out=xin[:], in_=src[:, :, st * ST : (st + 1) * ST],
                )
                return xin

            # critical consts first (first proj matmuls need these)
            nc.sync.dma_start(out=wq_sb[:], in_=WQ[:])
            nc.sync.dma_start(out=wk_sb[:], in_=WK[:])
            nc.sync.dma_start(out=wv_sb[:], in_=WV[:])
            nc.sync.dma_start(out=bq_sb[:], in_=BQ[:])
            nc.sync.dma_start(out=bk_sb[:], in_=BK[:])
            nc.sync.dma_start(out=bvb_sb[:], in_=BVB[:])
            prefetched[("q", 0)] = fetch(0, XQ, qin_p, "q")
            prefetched[("k", 0)] = fetch(0, XK, kin_p, "k")
            prefetched[("v", 0)] = fetch(0, XV, vin_p, "v")

            # bulky / later-needed consts
            nc.sync.dma_start(out=mask_sb[:], in_=MSK[:])
            nc.sync.dma_start(out=woR[:], in_=WOR[:])
            nc.sync.dma_start(out=onesP[:], in_=ONEP[:])

            # ones columns of the augmented v slots (col 64 per head slot)
            nc.gpsimd.memset(v_sb[:, :, SLOT - 1 : SLOT], 1.0)
            nc.gpsimd.memset(v_sb[:, :, 2 * SLOT - 1 : 2 * SLOT], 1.0)

            def get_in(st, name, src, in_pool):
                xin = prefetched.pop((name, st), None)
                if xin is None:
                    xin = fetch(st, src, in_pool, name)
                return xin

            v4 = v_sb.rearrange("p n (h c) -> p n h c", h=2)
            bvb4 = bvb_sb.rearrange("p (k h c) -> p k h c", k=4, h=2)

            def proj_units(st, xq, xk, xv):
                """Projection of s-tile st as schedulable PE work units."""
                state = {}

                def qk_part(xin, w_sb, b_sb, dst_ap, key, lo, hi):
                    def run():
                        if key not in state:
                            state[key] = psA.tile([128, ST], f32, tag="pp",
                                                  name=f"pp{key}{st}")
                        ps = state[key]
                        for t in range(lo, hi):
                            nc.tensor.matmul(
                                ps[:],
                                lhsT=w_sb[:, t, :],
                                rhs=xin[:, t, :],
                                start=(t == 0),
                                stop=(t == 7),
                            )
                        if hi == 8:
                            nc.vector.tensor_scalar_add(dst_ap, ps[:],
                                                        b_sb[:])
                    return run

                def v_part(qb):
                    def run():
                        if "v" not in state:
                            state["v"] = psA.tile([128, 4, 128], f32,
                                                  tag="pp", name=f"pv{st}")
                        pv = state["v"]
                        for t in range(8):
                            nc.tensor.matmul(
                                pv[:, qb, :],
                                lhsT=xv[:, t, qb * 128 : (qb + 1) * 128],
                                rhs=wv_sb[:, t, :],
                                start=(t == 0),
                                stop=(t == 7),
                            )
                        if qb == 3:
                            # bias-add + pack into augmented slots (skip the
                            # ones columns); DVE: GPSIMD cannot read PSUM
                            nc.vector.tensor_tensor(
                                out=v4[:, 4 * st : 4 * st + 4, :, 0:DK],
                                in0=pv.rearrange("p k (h c) -> p k h c", h=2),
                                in1=bvb4[:],
                                op=ADD,
                            )
                    return run

                qdst = qT_sb[:, st * ST : (st + 1) * ST]
                kdst = kT_sb[:, st * ST : (st + 1) * ST]
                return [
                    qk_part(xq, wq_sb, bq_sb, qdst, "q", 0, 4),
                    qk_part(xq, wq_sb, bq_sb, qdst, "q", 4, 8),
                    qk_part(xk, wk_sb, bk_sb, kdst, "k", 0, 4),
                    qk_part(xk, wk_sb, bk_sb, kdst, "k", 4, 8),
                    v_part(0), v_part(1), v_part(2), v_part(3),
                ]

            def attn(qt, filler):
                npr = 2 * qt + 2
                po = {}
                for h in (0, 1):
                    po[h] = psO.tile([65, ST], f32, tag="po",
                                     name=f"po{qt}_{h}")

                def attnv(pr, ex):
                    # attn @ V (+ones col => denominators in PSUM row 64)
                    for h in (0, 1):
                        for j in range(2):
                            kb = 2 * pr + j
                            rel = kb - 4 * qt
                            c0 = 128 * rel if rel > 0 else 0
                            nc.tensor.matmul(
                                po[h][:, c0:ST],
                                lhsT=v_sb[:, kb, h * SLOT : (h + 1) * SLOT],
                                rhs=ex[h][:, j * ST + c0 : (j + 1) * ST],
                                start=(pr == 0 and j == 0),
                                stop=(pr == npr - 1 and j == 1),
                            )

                prev = None  # (pr, ex) whose attn@V is still pending
                for pr in range(npr):
                    rels = [2 * pr + j - 4 * qt for j in (0, 1)]
                    ps = {}
                    for h in (0, 1):
                        ps[h] = psS.tile([128, 2 * ST], f32, tag="ps",
                                         name=f"ps{qt}_{h}_{pr}")
                    # scores: head-interleaved so the two K=64 matmuls run
                    # in different PE row-groups concurrently
                    for j in range(2):
                        kb = 2 * pr + j
                        rel = rels[j]
                        c0 = 128 * rel if rel > 0 else 0
                        for h in (0, 1):
                            nc.tensor.matmul(
                                ps[h][:, j * ST + c0 : (j + 1) * ST],
                                lhsT=kT_sb[64 * h : 64 * h + 64,
                                           kb * KB : (kb + 1) * KB],
                                rhs=qT_sb[64 * h : 64 * h + 64,
                                          qt * ST + c0 : (qt + 1) * ST],
                                start=True,
                                stop=True,
                            )
                    ex = {}
                    for h in (0, 1):
                        ex[h] = exp_p.tile([128, 2 * ST], bf16, tag="ex",
                                           name=f"ex{qt}_{h}_{pr}")
                        if rels[0] >= 2:  # steep diagonal pair: narrow exps
                            for j in range(2):
                                c0 = 128 * rels[j]
                                nc.scalar.activation(
                                    ex[h][:, j * ST + c0 : (j + 1) * ST],
                                    ps[h][:, j * ST + c0 : (j + 1) * ST],
                                    EXP, scale=0.125,
                                )
                        else:
                            nc.scalar.activation(ex[h][:], ps[h][:], EXP,
                                                 scale=0.125)
                    # zero the partial diagonal 128-bands (Pool engine)
                    for h in (0, 1):
                        for j in range(2):
                            rel = rels[j]
                            if rel >= 0:
                                b0 = j * ST + 128 * rel
                                nc.gpsimd.tensor_tensor(
                                    out=ex[h][:, b0 : b0 + 128],
                                    in0=ex[h][:, b0 : b0 + 128],
                                    in1=mask_sb[:],
                                    op=MULT,
                                )
                    # attn@V lags one pair so PE never stalls on this exp
                    if prev is not None:
                        attnv(*prev)
                    prev = (pr, ex)
                    # interleave pending proj/Wo units, spread evenly
                    filler(-(npr - pr))
                attnv(*prev)
                # normalize: woL[h*64:(h+1)*64, :] = po[h][0:64] / denom
                woL = wl_p.tile([128, ST], bf16, tag="wl", name=f"wl{qt}")
                for h in (0, 1):
                    den = den_p.tile([1, ST], f32r, tag="den",
                                     name=f"den{qt}_{h}")
                    nc.vector.tensor_copy(den[:], po[h][64:65, :])
                    pb = psA.tile([DK, ST], f32, tag="pp", name=f"pb{qt}_{h}")
                    nc.tensor.matmul(
                        pb[:], lhsT=onesP[:], rhs=den[:],
                        start=True, stop=True,
                    )
                    r_sb = rsb_p.tile([DK, ST], f32, tag="r",
                                      name=f"r{qt}_{h}")
                    nc.vector.reciprocal_approx_fast(out=r_sb[:], in_=pb[:])
                    nc.vector.tensor_tensor(
                        out=woL[64 * h : 64 * h + 64, :],
                        in0=po[h][0:64, :], in1=r_sb[:], op=MULT,
                    )
                woL_tiles[qt] = woL

            def wo_units(qt):
                """8 closures, each one output block of the Wo projection."""
                wl = woL_tiles.pop(qt)

                def unit(qb, nt):
                    def run():
                        q0 = qt * ST + qb * 128
                        pw = psA.tile([128, ST], f32, tag="pp",
                                      name=f"pw{qt}_{qb}_{nt}")
                        nc.tensor.matmul(
                            pw[:],
                            lhsT=wl[:, qb * 128 : (qb + 1) * 128],
                            rhs=woR[:, nt * ST : (nt + 1) * ST],
                            start=True, stop=True,
                        )
                        ob = oout_p.tile([128, ST], bf16, tag="ob",
                                         name=f"ob{qt}_{qb}_{nt}")
                        nc.vector.tensor_copy(ob[:], pw[:])
                        nc.sync.dma_start(
                            out=OUT[q0 : q0 + 128, nt * ST : (nt + 1) * ST],
                            in_=ob[:],
                        )
                    return run

                return [unit(qb, nt) for qb in range(4) for nt in range(2)]

            pending = []

            def filler(n):
                # n < 0: spread -> emit ceil(len/|n|); n > 0: emit up to n
                if n < 0:
                    n = -(len(pending) // n)
                for _ in range(min(n, len(pending))):
                    pending.pop(0)()

            for st in range(NST):
                if st == 0:
                    # first projection runs inline (nothing to overlap yet)
                    xq = get_in(0, "q", XQ, qin_p)
                    xk = get_in(0, "k", XK, kin_p)
                    xv = get_in(0, "v", XV, vin_p)
                    for u in proj_units(0, xq, xk, xv):
                        u()
                # leftover proj units for this tile must precede its scores
                filler(len(pending))
                if st + 1 < NST:
                    xq = fetch(st + 1, XQ, qin_p, "q")
                    xk = fetch(st + 1, XK, kin_p, "k")
                    xv = fetch(st + 1, XV, vin_p, "v")
                    pending.extend(proj_units(st + 1, xq, xk, xv))
                if st >= 1:
                    # wo for qt=st-1: drained by filler inside attn(st)
                    pending.extend(wo_units(st - 1))
                attn(st, filler)
            # drain the tail
            filler(len(pending))
            pending.extend(wo_units(NST - 1))
            filler(len(pending))

    nc.compile()
    return nc


def _prep_inputs(Q, K, V, Wq_w, Wq_b, Wk_w, Wk_b, Wv_w, Wv_b, Wo_w, Wo_b):
    bf = ml_dtypes.bfloat16
    f = np.float32

    def xprep(X):
        # [S, D] -> [128, 8, S]: x[p, t, s] = X[s, t*128+p]
        return np.ascontiguousarray(
            X[0].T.reshape(8, 128, S).transpose(1, 0, 2).astype(bf)
        )

    def wprep(Wslice):
        # Wslice [CD, D] -> [128, 8, CD]: w[p, t, c] = Wslice[c, t*128+p]
        return np.ascontiguousarray(
            Wslice.T.reshape(8, 128, CD).transpose(1, 0, 2).astype(bf)
        )

    XQp, XKp, XVp = xprep(Q), xprep(K), xprep(V)
    p = np.arange(KB)[:, None]
    fidx = np.arange(KB)[None, :]
    msk = np.where(p <= fidx, 1.0, 0.0).astype(bf)
    WoT = np.ascontiguousarray(Wo_w.T, dtype=f)  # [in, out]

    in_maps = []
    for c in range(NCORES):
        c0 = CD * c
        in_maps.append({
            "xq": XQp, "xk": XKp, "xv": XVp,
            "wq": wprep(Wq_w[c0 : c0 + CD, :]),
            "wk": wprep(Wk_w[c0 : c0 + CD, :]),
            "wv": wprep(Wv_w[c0 : c0 + CD, :]),
            "bq": np.ascontiguousarray(Wq_b[c0 : c0 + CD, None], dtype=f),
            "bk": np.ascontiguousarray(Wk_b[c0 : c0 + CD, None], dtype=f),
            "bvb": np.ascontiguousarray(
                np.broadcast_to(np.tile(Wv_b[c0 : c0 + CD], 4), (128, 512))
            ).astype(bf),
            "wor": np.ascontiguousarray(WoT[c0 : c0 + CD, :], dtype=bf),
            "msk": msk,
            "onep": np.ones((1, DK), f),
        })
    return in_maps


def _numpy_fallback(Q, K, V, Wq_w, Wq_b, Wk_w, Wk_b, Wv_w, Wv_b, Wo_w, Wo_b,
                    mask):
    q = (Q @ Wq_w.T + Wq_b).reshape(1, S, H, DK).transpose(0, 2, 1, 3)
    k = (K @ Wk_w.T + Wk_b).reshape(1, S, H, DK).transpose(0, 2, 1, 3)
    v = (V @ Wv_w.T + Wv_b).reshape(1, S, H, DK).transpose(0, 2, 1, 3)
    scores = np.einsum("bhqd,bhkd->bhqk", q, k) / np.sqrt(DK).astype(np.float32)
    scores = np.where(mask == 0, np.float32(-1e9), scores)
    scores -= scores.max(axis=-1, keepdims=True)
    e = np.exp(scores)
    attn = e / e.sum(axis=-1, keepdims=True)
    out = np.einsum("bhqk,bhkd->bhqd", attn, v)
    out = out.transpose(0, 2, 1, 3).reshape(1, S, D)
    return (out @ Wo_w.T + Wo_b).astype(np.float32)


def kernel(Q, K, V, Wq_w, Wq_b, Wk_w, Wk_b, Wv_w, Wv_b, Wo_w, Wo_b, mask,
           **run_kwargs):
    Q = np.asarray(Q); K = np.asarray(K); V = np.asarray(V)
    Wq_w = np.asarray(Wq_w); Wq_b = np.asarray(Wq_b)
    Wk_w = np.asarray(Wk_w); Wk_b = np.asarray(Wk_b)
    Wv_w = np.asarray(Wv_w); Wv_b = np.asarray(Wv_b)
    Wo_w = np.asarray(Wo_w); Wo_b = np.asarray(Wo_b)
    mask = np.asarray(mask)

    causal = np.array_equal(
        mask.reshape(S, S), np.tril(np.ones((S, S), mask.dtype))
    )
    if not causal:
        return _numpy_fallback(Q, K, V, Wq_w, Wq_b, Wk_w, Wk_b, Wv_w, Wv_b,
                               Wo_w, Wo_b, mask)

    from concourse.bass_utils import run_bass_kernel_spmd

    if _compiled[0] is None:
        _compiled[0] = _build()
    nc = _compiled[0]

    in_maps = _prep_inputs(Q, K, V, Wq_w, Wq_b, Wk_w, Wk_b, Wv_w, Wv_b,
                           Wo_w, Wo_b)
    res = run_bass_kernel_spmd(nc, in_maps, list(range(NCORES)), **run_kwargs)
    out = np.zeros((S, D), np.float32)
    for cres in res.results:
        out += np.asarray(cres["out"], dtype=np.float32)
    out += Wo_b.astype(np.float32)
    if run_kwargs:
        kernel.last_result = res
    return out.reshape(1, S, D).astype(np.float32)


# revision 22
# speedup vs baseline: 1.3514x; 1.0590x over previous
"""Multi-head attention (B=1, S=4096, D=1024, H=16, causal) on 8 Trainium2
NeuronCores.

Sharding: tensor-parallel over heads — each core owns 2 heads (128 of the
1024 projection dims). Wq/Wk/Wv are split column-wise, Wo row-wise; each
core computes a full [S, D] partial of the output projection (bf16) and the
all-reduce is done on the host by summing the 8 partials (+ Wo_b once).

All matmul operands are bf16 (f32 PSUM accumulation): same 1 cycle/row PE
rate as f32r but FWL-eligible weight loads, half the DMA/SBUF traffic, and
2x DVE modes where applicable.

Per-core device kernel:
  qT/kT projections produce [c=128, S] bf16 directly (contract D streams
  from host-pretransposed inputs); the two heads live on partition halves
  0-63 / 64-127 so the per-head score matmuls (contract 64) auto-derive
  PE row tiles (0,0)/(64,0) and run concurrently in the array.
  v is projected directly into [s, c] layout (x-subtile stationary) and
  bias-added into an augmented [s, 65]-per-head slot (ones column => softmax
  denominator falls out of the attn@V matmul as PSUM row 64).
  Scores are computed transposed (scoresT[k, q]) so softmax exp is the PSUM
  eviction (ACT, scale=1/8, bf16 out); the partial diagonal 128-bands are
  zeroed by a Pool-engine mask multiply; fully-masked blocks are skipped.
  Normalization (1/denom) is broadcast across partitions with a K=1 ones
  matmul; the normalized bf16 [c, q] tiles for both heads land in one
  [128, q] tile so the final Wo projection is a single K=128 matmul per
  output block, interleaved into the next attention tile's PE stream.
"""

import numpy as np
import ml_dtypes

D = 1024
H = 16
DK = D // H  # 64
S = 4096
NCORES = 8
CD = 128          # c-dims (2 heads) per core
ST = 512          # s/q tile
NST = S // ST     # 8
KB = 128          # k block
NKB = S // KB     # 32
SLOT = 128        # v_sb cols per head per k-block (64 dims + 64 ones)

_compiled = [None]


def _build():
    import concourse.bacc as bacc
    import concourse.mybir as mybir
    import concourse.tile as tile

    f32 = mybir.dt.float32
    f32r = mybir.dt.float32r
    bf16 = mybir.dt.bfloat16
    EXP = mybir.ActivationFunctionType.Exp
    MULT = mybir.AluOpType.mult
    ADD = mybir.AluOpType.add

    nc = bacc.Bacc(None, target_bir_lowering=False)

    XQ = nc.dram_tensor("xq", [128, 8, S], bf16, kind="ExternalInput")
    XK = nc.dram_tensor("xk", [128, 8, S], bf16, kind="ExternalInput")
    XV = nc.dram_tensor("xv", [128, 8, S], bf16, kind="ExternalInput")
    WQ = nc.dram_tensor("wq", [128, 8, CD], bf16, kind="ExternalInput")
    WK = nc.dram_tensor("wk", [128, 8, CD], bf16, kind="ExternalInput")
    WV = nc.dram_tensor("wv", [128, 8, CD], bf16, kind="ExternalInput")
    BQ = nc.dram_tensor("bq", [CD, 1], f32, kind="ExternalInput")
    BK = nc.dram_tensor("bk", [CD, 1], f32, kind="ExternalInput")
    BVB = nc.dram_tensor("bvb", [128, 512], bf16, kind="ExternalInput")
    WOR = nc.dram_tensor("wor", [CD, D], bf16, kind="ExternalInput")
    MSK = nc.dram_tensor("msk", [KB, KB], bf16, kind="ExternalInput")
    OUT = nc.dram_tensor("out", [S, D], bf16, kind="ExternalOutput")

    with tile.TileContext(nc) as tc:
        with (
            tc.tile_pool(name="const", bufs=1) as const,
            tc.tile_pool(name="qin", bufs=2) as qin_p,
            tc.tile_pool(name="kin", bufs=2) as kin_p,
            tc.tile_pool(name="vin", bufs=2) as vin_p,
            tc.tile_pool(name="expp", bufs=6) as exp_p,
            tc.tile_pool(name="rsb", bufs=4) as rsb_p,
            tc.tile_pool(name="wlp", bufs=3) as wl_p,
            tc.tile_pool(name="oout", bufs=4) as oout_p,
            tc.tile_pool(name="psA", bufs=2, space="PSUM") as psA,
            tc.tile_pool(name="psS", bufs=2, space="PSUM") as psS,
            tc.tile_pool(name="psO", bufs=2, space="PSUM") as psO,
        ):
            # ---- static SBUF tensors ----
            qT_sb = const.tile([CD, S], bf16, tag="qT")
            kT_sb = const.tile([CD, S], bf16, tag="kT")
            v_sb = const.tile([128, NKB, 2 * SLOT], bf16, tag="vsb")

            wq_sb = const.tile([128, 8, CD], bf16, tag="wq")
            wk_sb = const.tile([128, 8, CD], bf16, tag="wk")
            wv_sb = const.tile([128, 8, CD], bf16, tag="wv")
            woR = const.tile([CD, D], bf16, tag="woR")
            mask_sb = const.tile([KB, KB], bf16, tag="mask")
            bq_sb = const.tile([CD, 1], f32, tag="bq")
            bk_sb = const.tile([CD, 1], f32, tag="bk")
            bvb_sb = const.tile([128, 512], bf16, tag="bvb")

            woL_tiles = {}
            prefetched = {}

            def fetch(st, src, in_pool, name):
                xin = in_pool.tile([128, 8, ST], bf16, tag="xin",
                                   name=f"xin_{name}{st}")
                for g in range(2):
                    nc.sync.dma_start(
                        out=xin[:, 4 * g : 4 * g + 4, :],
                        in_=src[:, 4 * g : 4 * g + 4,
                                st * ST : (st + 1) * ST],
                    )
                return xin

            # critical consts first (first proj matmuls need these)
            nc.sync.dma_start(out=wq_sb[:], in_=WQ[:])
            nc.sync.dma_start(out=wk_sb[:], in_=WK[:])
            nc.sync.dma_start(out=wv_sb[:], in_=WV[:])
            nc.sync.dma_start(out=bq_sb[:], in_=BQ[:])
            nc.sync.dma_start(out=bk_sb[:], in_=BK[:])
            nc.sync.dma_start(out=bvb_sb[:], in_=BVB[:])
            prefetched[("q", 0)] = fetch(0, XQ, qin_p, "q")
            prefetched[("k", 0)] = fetch(0, XK, kin_p, "k")
            prefetched[("v", 0)] = fetch(0, XV, vin_p, "v")

            # bulky / later-needed consts
            nc.sync.dma_start(out=mask_sb[:], in_=MSK[:])
            nc.sync.dma_start(out=woR[:], in_=WOR[:])

            # ones blocks of the augmented v slots (cols 0-63 per head
            # slot => attn@V lands denominators on PSUM partitions 0-63,
            # numerators on 64-127)
            nc.gpsimd.memset(v_sb[:, :, 0:DK], 1.0)
            nc.gpsimd.memset(v_sb[:, :, SLOT : SLOT + DK], 1.0)

            def get_in(st, name, src, in_pool):
                xin = prefetched.pop((name, st), None)
                if xin is None:
                    xin = fetch(st, src, in_pool, name)
                return xin

            v4 = v_sb.rearrange("p n (h c) -> p n h c", h=2)
            bvb4 = bvb_sb.rearrange("p (k h c) -> p k h c", k=4, h=2)

            def proj_units(st, xq, xk, xv):
                """Projection of s-tile st as schedulable PE work units."""
                state = {}

                def qk_part(xin, w_sb, b_sb, dst_ap, key, lo, hi):
                    def run():
                        if key not in state:
                            state[key] = psA.tile([128, ST], f32, tag="pp",
                                                  name=f"pp{key}{st}")
                        ps = state[key]
                        for t in range(lo, hi):
                            nc.tensor.matmul(
                                ps[:],
                                lhsT=w_sb[:, t, :],
                                rhs=xin[:, t, :],
                                start=(t == 0),
                                stop=(t == 7),
                            )
                        if hi == 8:
                            nc.vector.tensor_scalar_add(dst_ap, ps[:],
                                                        b_sb[:])
                    return run

                def v_part(qb):
                    def run():
                        if "v" not in state:
                            state["v"] = psA.tile([128, 4, 128], f32,
                                                  tag="pp", name=f"pv{st}")
                        pv = state["v"]
                        for t in range(8):
                            nc.tensor.matmul(
                                pv[:, qb, :],
                                lhsT=xv[:, t, qb * 128 : (qb + 1) * 128],
                                rhs=wv_sb[:, t, :],
                                start=(t == 0),
                                stop=(t == 7),
                            )
                        if qb == 3:
                            # bias-add + pack into augmented slots (skip the
                            # ones columns); DVE: GPSIMD cannot read PSUM
                            nc.vector.tensor_tensor(
                                out=v4[:, 4 * st : 4 * st + 4, :, DK:SLOT],
                                in0=pv.rearrange("p k (h c) -> p k h c", h=2),
                                in1=bvb4[:],
                                op=ADD,
                            )
                    return run

                qdst = qT_sb[:, st * ST : (st + 1) * ST]
                kdst = kT_sb[:, st * ST : (st + 1) * ST]
                return [
                    qk_part(xq, wq_sb, bq_sb, qdst, "q", 0, 4),
                    qk_part(xq, wq_sb, bq_sb, qdst, "q", 4, 8),
                    qk_part(xk, wk_sb, bk_sb, kdst, "k", 0, 4),
                    qk_part(xk, wk_sb, bk_sb, kdst, "k", 4, 8),
                    v_part(0), v_part(1), v_part(2), v_part(3),
                ]

            def attn(qt, filler):
                npr = 2 * qt + 2
                po = {}
                for h in (0, 1):
                    po[h] = psO.tile([128, ST], f32, tag="po",
                                     name=f"po{qt}_{h}")

                def attnv(pr, ex):
                    # attn @ V (+ones col => denominators in PSUM row 64)
                    for h in (0, 1):
                        for j in range(2):
                            kb = 2 * pr + j
                            rel = kb - 4 * qt
                            c0 = 128 * rel if rel > 0 else 0
                            nc.tensor.matmul(
                                po[h][:, c0:ST],
                                lhsT=v_sb[:, kb, h * SLOT : (h + 1) * SLOT],
                                rhs=ex[h][:, j * ST + c0 : (j + 1) * ST],
                                start=(pr == 0 and j == 0),
                                stop=(pr == npr - 1 and j == 1),
                            )

                prev = None  # (pr, ex) whose attn@V is still pending
                for pr in range(npr):
                    rels = [2 * pr + j - 4 * qt for j in (0, 1)]
                    ps = {}
                    for h in (0, 1):
                        ps[h] = psS.tile([128, 2 * ST], f32, tag="ps",
                                         name=f"ps{qt}_{h}_{pr}")
                    # scores: head-interleaved so the two K=64 matmuls run
                    # in different PE row-groups concurrently
                    for j in range(2):
                        kb = 2 * pr + j
                        rel = rels[j]
                        c0 = 128 * rel if rel > 0 else 0
                        for h in (0, 1):
                            nc.tensor.matmul(
                                ps[h][:, j * ST + c0 : (j + 1) * ST],
                                lhsT=kT_sb[64 * h : 64 * h + 64,
                                           kb * KB : (kb + 1) * KB],
                                rhs=qT_sb[64 * h : 64 * h + 64,
                                          qt * ST + c0 : (qt + 1) * ST],
                                start=True,
                                stop=True,
                            )
                    ex = {}
                    for h in (0, 1):
                        ex[h] = exp_p.tile([128, 2 * ST], bf16, tag="ex",
                                           name=f"ex{qt}_{h}_{pr}")
                        if rels[0] >= 2:  # steep diagonal pair: narrow exps
                            for j in range(2):
                                c0 = 128 * rels[j]
                                nc.scalar.activation(
                                    ex[h][:, j * ST + c0 : (j + 1) * ST],
                                    ps[h][:, j * ST + c0 : (j + 1) * ST],
                                    EXP, scale=0.125,
                                )
                        else:
                            nc.scalar.activation(ex[h][:], ps[h][:], EXP,
                                                 scale=0.125)
                    # zero the partial diagonal 128-bands (Pool engine)
                    for h in (0, 1):
                        for j in range(2):
                            rel = rels[j]
                            if rel >= 0:
                                b0 = j * ST + 128 * rel
                                nc.gpsimd.tensor_tensor(
                                    out=ex[h][:, b0 : b0 + 128],
                                    in0=ex[h][:, b0 : b0 + 128],
                                    in1=mask_sb[:],
                                    op=MULT,
                                )
                    # attn@V lags one pair so PE never stalls on this exp
                    if prev is not None:
                        attnv(*prev)
                    prev = (pr, ex)
                    # interleave pending proj/Wo units, spread evenly
                    filler(-(npr - pr))
                attnv(*prev)
                # normalize: woL[h*64:(h+1)*64, :] = po[h][0:64] / denom
                woL = wl_p.tile([128, ST], bf16, tag="wl", name=f"wl{qt}")
                for h in (0, 1):
                    r_sb = rsb_p.tile([DK, ST], f32, tag="r",
                                      name=f"r{qt}_{h}")
                    nc.vector.reciprocal_approx_fast(out=r_sb[:],
                                                     in_=po[h][0:64, :])
                    nc.vector.tensor_tensor(
                        out=woL[64 * h : 64 * h + 64, :],
                        in0=po[h][64:128, :], in1=r_sb[:], op=MULT,
                    )
                woL_tiles[qt] = woL

            def wo_units(qt):
                """8 closures, each one output block of the Wo projection."""
                wl = woL_tiles.pop(qt)

                def unit(qb, nt):
                    def run():
                        q0 = qt * ST + qb * 128
                        pw = psA.tile([128, ST], f32, tag="pp",
                                      name=f"pw{qt}_{qb}_{nt}")
                        nc.tensor.matmul(
                            pw[:],
                            lhsT=wl[:, qb * 128 : (qb + 1) * 128],
                            rhs=woR[:, nt * ST : (nt + 1) * ST],
                            start=True, stop=True,
                        )
                        ob = oout_p.tile([128, ST], bf16, tag="ob",
                                         name=f"ob{qt}_{qb}_{nt}")
                        nc.vector.tensor_copy(ob[:], pw[:])
                        nc.sync.dma_start(
                            out=OUT[q0 : q0 + 128, nt * ST : (nt + 1) * ST],
                            in_=ob[:],
                        )
                    return run

                return [unit(qb, nt) for qb in range(4) for nt in range(2)]

            pending = []

            def filler(n):
                # n < 0: spread -> emit ceil(len/|n|); n > 0: emit up to n
                if n < 0:
                    n = -(len(pending) // n)
                for _ in range(min(n, len(pending))):
                    pending.pop(0)()

            for st in range(NST):
                if st == 0:
                    # first projection runs inline (nothing to overlap yet)
                    xq = get_in(0, "q", XQ, qin_p)
                    xk = get_in(0, "k", XK, kin_p)
                    xv = get_in(0, "v", XV, vin_p)
                    for u in proj_units(0, xq, xk, xv):
                        u()
                # leftover proj units for this tile must precede its scores
                filler(len(pending))
                if st + 1 < NST:
                    xq = fetch(st + 1, XQ, qin_p, "q")
                    xk = fetch(st + 1, XK, kin_p, "k")
                    xv = fetch(st + 1, XV, vin_p, "v")
                    pending.extend(proj_units(st + 1, xq, xk, xv))
                if st >= 1:
                    # wo for qt=st-1: drained by filler inside attn(st)
                    pending.extend(wo_units(st - 1))
                attn(st, filler)
            # drain the tail
            filler(len(pending))
            pending.extend(wo_units(NST - 1))
            filler(len(pending))

    nc.compile()
    return nc


def _prep_inputs(Q, K, V, Wq_w, Wq_b, Wk_w, Wk_b, Wv_w, Wv_b, Wo_w, Wo_b):
    bf = ml_dtypes.bfloat16
    f = np.float32

    def xprep(X):
        # [S, D] -> [128, 8, S]: x[p, t, s] = X[s, t*128+p]
        return np.ascontiguousarray(
            X[0].T.reshape(8, 128, S).transpose(1, 0, 2).astype(bf)
        )

    def wprep(Wslice):
        # Wslice [CD, D] -> [128, 8, CD]: w[p, t, c] = Wslice[c, t*128+p]
        return np.ascontiguousarray(
            Wslice.T.reshape(8, 128, CD).transpose(1, 0, 2).astype(bf)
        )

    XQp, XKp, XVp = xprep(Q), xprep(K), xprep(V)
    p = np.arange(KB)[:, None]
    fidx = np.arange(KB)[None, :]
    msk = np.where(p <= fidx, 1.0, 0.0).astype(bf)
    WoT = np.ascontiguousarray(Wo_w.T, dtype=f)  # [in, out]

    in_maps = []
    for c in range(NCORES):
        c0 = CD * c
        in_maps.append({
            "xq": XQp, "xk": XKp, "xv": XVp,
            "wq": wprep(Wq_w[c0 : c0 + CD, :]),
            "wk": wprep(Wk_w[c0 : c0 + CD, :]),
            "wv": wprep(Wv_w[c0 : c0 + CD, :]),
            "bq": np.ascontiguousarray(Wq_b[c0 : c0 + CD, None], dtype=f),
            "bk": np.ascontiguousarray(Wk_b[c0 : c0 + CD, None], dtype=f),
            "bvb": np.ascontiguousarray(
                np.broadcast_to(np.tile(Wv_b[c0 : c0 + CD], 4), (128, 512))
            ).astype(bf),
            "wor": np.ascontiguousarray(WoT[c0 : c0 + CD, :], dtype=bf),
            "msk": msk,
        })
    return in_maps


def _numpy_fallback(Q, K, V, Wq_w, Wq_b, Wk_w, Wk_b, Wv_w, Wv_b, Wo_w, Wo_b,
                    mask):
    q = (Q @ Wq_w.T + Wq_b).reshape(1, S, H, DK).transpose(0, 2, 1, 3)
    k = (K @ Wk_w.T + Wk_b).reshape(1, S, H, DK).transpose(0, 2, 1, 3)
    v = (V @ Wv_w.T + Wv_b).reshape(1, S, H, DK).transpose(0, 2, 1, 3)
    scores = np.einsum("bhqd,bhkd->bhqk", q, k) / np.sqrt(DK).astype(np.float32)
    scores = np.where(mask == 0, np.float32(-1e9), scores)
    scores -= scores.max(axis=-1, keepdims=True)
    e = np.exp(scores)
    attn = e / e.sum(axis=-1, keepdims=True)
    out = np.einsum("bhqk,bhkd->bhqd", attn, v)
    out = out.transpose(0, 2, 1, 3).reshape(1, S, D)
    return (out @ Wo_w.T + Wo_b).astype(np.float32)


def kernel(Q, K, V, Wq_w, Wq_b, Wk_w, Wk_b, Wv_w, Wv_b, Wo_w, Wo_b, mask,
           **run_kwargs):
    Q = np.asarray(Q); K = np.asarray(K); V = np.asarray(V)
    Wq_w = np.asarray(Wq_w); Wq_b = np.asarray(Wq_b)
    Wk_w = np.asarray(Wk_w); Wk_b = np.asarray(Wk_b)
    Wv_w = np.asarray(Wv_w); Wv_b = np.asarray(Wv_b)
    Wo_w = np.asarray(Wo_w); Wo_b = np.asarray(Wo_b)
    mask = np.asarray(mask)

    causal = np.array_equal(
        mask.reshape(S, S), np.tril(np.ones((S, S), mask.dtype))
    )
    if not causal:
        return _numpy_fallback(Q, K, V, Wq_w, Wq_b, Wk_w, Wk_b, Wv_w, Wv_b,
                               Wo_w, Wo_b, mask)

    from concourse.bass_utils import run_bass_kernel_spmd

    if _compiled[0] is None:
        _compiled[0] = _build()
    nc = _compiled[0]

    in_maps = _prep_inputs(Q, K, V, Wq_w, Wq_b, Wk_w, Wk_b, Wv_w, Wv_b,
                           Wo_w, Wo_b)
    res = run_bass_kernel_spmd(nc, in_maps, list(range(NCORES)), **run_kwargs)
    out = np.zeros((S, D), np.float32)
    for cres in res.results:
        out += np.asarray(cres["out"], dtype=np.float32)
    out += Wo_b.astype(np.float32)
    if run_kwargs:
        kernel.last_result = res
    return out.reshape(1, S, D).astype(np.float32)


# revision 23
# speedup vs baseline: 1.4053x; 1.0399x over previous
"""Multi-head attention (B=1, S=4096, D=1024, H=16, causal) on 8 Trainium2
NeuronCores.

Sharding: tensor-parallel over heads — each core owns 2 heads (128 of the
1024 projection dims). Wq/Wk/Wv are split column-wise, Wo row-wise; each
core computes a full [S, D] partial of the output projection (bf16) and the
all-reduce is done on the host by summing the 8 partials (+ Wo_b once).

All matmul operands are bf16 (f32 PSUM accumulation): same 1 cycle/row PE
rate as f32r but FWL-eligible weight loads, half the DMA/SBUF traffic, and
2x DVE modes where applicable.

Per-core device kernel:
  qT/kT projections produce [c=128, S] bf16 directly (contract D streams
  from host-pretransposed inputs); the two heads live on partition halves
  0-63 / 64-127 so the per-head score matmuls (contract 64) auto-derive
  PE row tiles (0,0)/(64,0) and run concurrently in the array.
  v is projected directly into [s, c] layout (x-subtile stationary) and
  bias-added into an augmented [s, 65]-per-head slot (ones column => softmax
  denominator falls out of the attn@V matmul as PSUM row 64).
  Scores are computed transposed (scoresT[k, q]) so softmax exp is the PSUM
  eviction (ACT, scale=1/8, bf16 out); the partial diagonal 128-bands are
  zeroed by a Pool-engine mask multiply; fully-masked blocks are skipped.
  Normalization (1/denom) is broadcast across partitions with a K=1 ones
  matmul; the normalized bf16 [c, q] tiles for both heads land in one
  [128, q] tile so the final Wo projection is a single K=128 matmul per
  output block, interleaved into the next attention tile's PE stream.
"""

import numpy as np
import ml_dtypes

D = 1024
H = 16
DK = D // H  # 64
S = 4096
NCORES = 8
CD = 128          # c-dims (2 heads) per core
ST = 512          # s/q tile
NST = S // ST     # 8
KB = 128          # k block
NKB = S // KB     # 32
SLOT = 128        # v_sb cols per head per k-block (64 dims + 64 ones)

_compiled = [None]


def _build():
    import concourse.bacc as bacc
    import concourse.mybir as mybir
    import concourse.tile as tile

    f32 = mybir.dt.float32
    f32r = mybir.dt.float32r
    bf16 = mybir.dt.bfloat16
    EXP = mybir.ActivationFunctionType.Exp
    MULT = mybir.AluOpType.mult
    ADD = mybir.AluOpType.add

    nc = bacc.Bacc(None, target_bir_lowering=False)

    XQ = nc.dram_tensor("xq", [128, 8, S], bf16, kind="ExternalInput")
    XK = nc.dram_tensor("xk", [128, 8, S], bf16, kind="ExternalInput")
    XV = nc.dram_tensor("xv", [128, 8, S], bf16, kind="ExternalInput")
    WQ = nc.dram_tensor("wq", [128, 8, CD], bf16, kind="ExternalInput")
    WK = nc.dram_tensor("wk", [128, 8, CD], bf16, kind="ExternalInput")
    WV = nc.dram_tensor("wv", [128, 8, CD], bf16, kind="ExternalInput")
    BQ = nc.dram_tensor("bq", [CD, 1], f32, kind="ExternalInput")
    BK = nc.dram_tensor("bk", [CD, 1], f32, kind="ExternalInput")
    BVB = nc.dram_tensor("bvb", [128, 512], bf16, kind="ExternalInput")
    WOR = nc.dram_tensor("wor", [CD, D], bf16, kind="ExternalInput")
    MSK = nc.dram_tensor("msk", [KB, KB], bf16, kind="ExternalInput")
    OUT = nc.dram_tensor("out", [S, D], bf16, kind="ExternalOutput")

    with tile.TileContext(nc) as tc:
        with (
            tc.tile_pool(name="const", bufs=1) as const,
            tc.tile_pool(name="qin", bufs=3) as qin_p,
            tc.tile_pool(name="kin", bufs=3) as kin_p,
            tc.tile_pool(name="vin", bufs=3) as vin_p,
            tc.tile_pool(name="expp", bufs=6) as exp_p,
            tc.tile_pool(name="rsb", bufs=4) as rsb_p,
            tc.tile_pool(name="wlp", bufs=3) as wl_p,
            tc.tile_pool(name="oout", bufs=4) as oout_p,
            tc.tile_pool(name="psA", bufs=2, space="PSUM") as psA,
            tc.tile_pool(name="psS", bufs=2, space="PSUM") as psS,
            tc.tile_pool(name="psO", bufs=2, space="PSUM") as psO,
        ):
            # ---- static SBUF tensors ----
            qT_sb = const.tile([CD, S], bf16, tag="qT")
            kT_sb = const.tile([CD, S], bf16, tag="kT")
            v_sb = const.tile([128, NKB, 2 * SLOT], bf16, tag="vsb")

            wq_sb = const.tile([128, 8, CD], bf16, tag="wq")
            wk_sb = const.tile([128, 8, CD], bf16, tag="wk")
            wv_sb = const.tile([128, 8, CD], bf16, tag="wv")
            woR = const.tile([CD, D], bf16, tag="woR")
            mask_sb = const.tile([KB, KB], bf16, tag="mask")
            bq_sb = const.tile([CD, 1], f32, tag="bq")
            bk_sb = const.tile([CD, 1], f32, tag="bk")
            bvb_sb = const.tile([128, 512], bf16, tag="bvb")

            woL_tiles = {}
            prefetched = {}

            def fetch(st, src, in_pool, name):
                xin = in_pool.tile([128, 8, ST], bf16, tag="xin",
                                   name=f"xin_{name}{st}")
                for g in range(2):
                    nc.sync.dma_start(
                        out=xin[:, 4 * g : 4 * g + 4, :],
                        in_=src[:, 4 * g : 4 * g + 4,
                                st * ST : (st + 1) * ST],
                    )
                return xin

            # critical consts first (first proj matmuls need these)
            nc.sync.dma_start(out=wq_sb[:], in_=WQ[:])
            nc.sync.dma_start(out=wk_sb[:], in_=WK[:])
            nc.sync.dma_start(out=wv_sb[:], in_=WV[:])
            nc.sync.dma_start(out=bq_sb[:], in_=BQ[:])
            nc.sync.dma_start(out=bk_sb[:], in_=BK[:])
            nc.sync.dma_start(out=bvb_sb[:], in_=BVB[:])
            prefetched[("q", 0)] = fetch(0, XQ, qin_p, "q")
            prefetched[("k", 0)] = fetch(0, XK, kin_p, "k")
            prefetched[("v", 0)] = fetch(0, XV, vin_p, "v")

            # bulky / later-needed consts
            nc.sync.dma_start(out=mask_sb[:], in_=MSK[:])
            nc.sync.dma_start(out=woR[:], in_=WOR[:])

            # ones blocks of the augmented v slots (cols 0-63 per head
            # slot => attn@V lands denominators on PSUM partitions 0-63,
            # numerators on 64-127)
            nc.gpsimd.memset(v_sb[:, :, 0:DK], 1.0)
            nc.gpsimd.memset(v_sb[:, :, SLOT : SLOT + DK], 1.0)

            def get_in(st, name, src, in_pool):
                xin = prefetched.pop((name, st), None)
                if xin is None:
                    xin = fetch(st, src, in_pool, name)
                return xin

            v4 = v_sb.rearrange("p n (h c) -> p n h c", h=2)
            bvb4 = bvb_sb.rearrange("p (k h c) -> p k h c", k=4, h=2)

            def proj_units(st, xq, xk, xv):
                """Projection of s-tile st as schedulable PE work units."""
                state = {}

                def qk_part(xin, w_sb, b_sb, dst_ap, key, lo, hi):
                    def run():
                        if key not in state:
                            state[key] = psA.tile([128, ST], f32, tag="pp",
                                                  name=f"pp{key}{st}")
                        ps = state[key]
                        for t in range(lo, hi):
                            nc.tensor.matmul(
                                ps[:],
                                lhsT=w_sb[:, t, :],
                                rhs=xin[:, t, :],
                                start=(t == 0),
                                stop=(t == 7),
                            )
                        if hi == 8:
                            nc.vector.tensor_scalar_add(dst_ap, ps[:],
                                                        b_sb[:])
                    return run

                def v_part(qb):
                    def run():
                        if "v" not in state:
                            state["v"] = psA.tile([128, 4, 128], f32,
                                                  tag="pp", name=f"pv{st}")
                        pv = state["v"]
                        for t in range(8):
                            nc.tensor.matmul(
                                pv[:, qb, :],
                                lhsT=xv[:, t, qb * 128 : (qb + 1) * 128],
                                rhs=wv_sb[:, t, :],
                                start=(t == 0),
                                stop=(t == 7),
                            )
                        if qb == 3:
                            # bias-add + pack into augmented slots (skip the
                            # ones columns); DVE: GPSIMD cannot read PSUM
                            nc.vector.tensor_tensor(
                                out=v4[:, 4 * st : 4 * st + 4, :, DK:SLOT],
                                in0=pv.rearrange("p k (h c) -> p k h c", h=2),
                                in1=bvb4[:],
                                op=ADD,
                            )
                    return run

                qdst = qT_sb[:, st * ST : (st + 1) * ST]
                kdst = kT_sb[:, st * ST : (st + 1) * ST]
                return [
                    qk_part(xq, wq_sb, bq_sb, qdst, "q", 0, 4),
                    qk_part(xq, wq_sb, bq_sb, qdst, "q", 4, 8),
                    qk_part(xk, wk_sb, bk_sb, kdst, "k", 0, 4),
                    qk_part(xk, wk_sb, bk_sb, kdst, "k", 4, 8),
                    v_part(0), v_part(1), v_part(2), v_part(3),
                ]

            def attn(qt, filler):
                npr = 2 * qt + 2
                po = {}
                for h in (0, 1):
                    po[h] = psO.tile([128, ST], f32, tag="po",
                                     name=f"po{qt}_{h}")

                def attnv(pr, ex):
                    # attn @ V (+ones col => denominators in PSUM row 64)
                    for h in (0, 1):
                        for j in range(2):
                            kb = 2 * pr + j
                            rel = kb - 4 * qt
                            c0 = 128 * rel if rel > 0 else 0
                            nc.tensor.matmul(
                                po[h][:, c0:ST],
                                lhsT=v_sb[:, kb, h * SLOT : (h + 1) * SLOT],
                                rhs=ex[h][:, j * ST + c0 : (j + 1) * ST],
                                start=(pr == 0 and j == 0),
                                stop=(pr == npr - 1 and j == 1),
                            )

                prev = None  # (pr, ex) whose attn@V is still pending
                for pr in range(npr):
                    rels = [2 * pr + j - 4 * qt for j in (0, 1)]
                    ps = {}
                    for h in (0, 1):
                        ps[h] = psS.tile([128, 2 * ST], f32, tag="ps",
                                         name=f"ps{qt}_{h}_{pr}")
                    # scores: head-interleaved so the two K=64 matmuls run
                    # in different PE row-groups concurrently
                    for j in range(2):
                        kb = 2 * pr + j
                        rel = rels[j]
                        c0 = 128 * rel if rel > 0 else 0
                        for h in (0, 1):
                            nc.tensor.matmul(
                                ps[h][:, j * ST + c0 : (j + 1) * ST],
                                lhsT=kT_sb[64 * h : 64 * h + 64,
                                           kb * KB : (kb + 1) * KB],
                                rhs=qT_sb[64 * h : 64 * h + 64,
                                          qt * ST + c0 : (qt + 1) * ST],
                                start=True,
                                stop=True,
                            )
                    ex = {}
                    for h in (0, 1):
                        ex[h] = exp_p.tile([128, 2 * ST], bf16, tag="ex",
                                           name=f"ex{qt}_{h}_{pr}")
                        if rels[0] >= 2:  # steep diagonal pair: narrow exps
                            for j in range(2):
                                c0 = 128 * rels[j]
                                nc.scalar.activation(
                                    ex[h][:, j * ST + c0 : (j + 1) * ST],
                                    ps[h][:, j * ST + c0 : (j + 1) * ST],
                                    EXP, scale=0.125,
                                )
                        else:
                            nc.scalar.activation(ex[h][:], ps[h][:], EXP,
                                                 scale=0.125)
                    # zero the partial diagonal 128-bands (Pool engine)
                    for h in (0, 1):
                        for j in range(2):
                            rel = rels[j]
                            if rel >= 0:
                                b0 = j * ST + 128 * rel
                                nc.gpsimd.tensor_tensor(
                                    out=ex[h][:, b0 : b0 + 128],
                                    in0=ex[h][:, b0 : b0 + 128],
                                    in1=mask_sb[:],
                                    op=MULT,
                                )
                    # attn@V lags one pair so PE never stalls on this exp
                    if prev is not None:
                        attnv(*prev)
                    prev = (pr, ex)
                    # interleave pending proj/Wo units, spread evenly
                    filler(-(npr - pr))
                attnv(*prev)
                # normalize: woL[h*64:(h+1)*64, :] = po[h][0:64] / denom
                woL = wl_p.tile([128, ST], bf16, tag="wl", name=f"wl{qt}")
                for h in (0, 1):
                    r_sb = rsb_p.tile([DK, ST], f32, tag="r",
                                      name=f"r{qt}_{h}")
                    nc.vector.reciprocal_approx_fast(out=r_sb[:],
                                                     in_=po[h][0:64, :])
                    nc.vector.tensor_tensor(
                        out=woL[64 * h : 64 * h + 64, :],
                        in0=po[h][64:128, :], in1=r_sb[:], op=MULT,
                    )
                woL_tiles[qt] = woL

            def wo_units(qt):
                """8 closures, each one output block of the Wo projection."""
                wl = woL_tiles.pop(qt)

                def unit(qb, nt):
                    def run():
                        q0 = qt * ST + qb * 128
                        pw = psA.tile([128, ST], f32, tag="pp",
                                      name=f"pw{qt}_{qb}_{nt}")
                        nc.tensor.matmul(
                            pw[:],
                            lhsT=wl[:, qb * 128 : (qb + 1) * 128],
                            rhs=woR[:, nt * ST : (nt + 1) * ST],
                            start=True, stop=True,
                        )
                        ob = oout_p.tile([128, ST], bf16, tag="ob",
                                         name=f"ob{qt}_{qb}_{nt}")
                        nc.vector.tensor_copy(ob[:], pw[:])
                        nc.sync.dma_start(
                            out=OUT[q0 : q0 + 128, nt * ST : (nt + 1) * ST],
                            in_=ob[:],
                        )
                    return run

                return [unit(qb, nt) for qb in range(4) for nt in range(2)]

            pending = []

            def filler(n):
                # n < 0: spread -> emit ceil(len/|n|); n > 0: emit up to n
                if n < 0:
                    n = -(len(pending) // n)
                for _ in range(min(n, len(pending))):
                    pending.pop(0)()

            for st in range(NST):
                if st == 0:
                    # first projection runs inline (nothing to overlap yet)
                    xq = get_in(0, "q", XQ, qin_p)
                    xk = get_in(0, "k", XK, kin_p)
                    xv = get_in(0, "v", XV, vin_p)
                    for u in proj_units(0, xq, xk, xv):
                        u()
                # leftover proj units for this tile must precede its scores
                filler(len(pending))
                if st + 1 < NST:
                    xq = fetch(st + 1, XQ, qin_p, "q")
                    xk = fetch(st + 1, XK, kin_p, "k")
                    xv = fetch(st + 1, XV, vin_p, "v")
                    pending.extend(proj_units(st + 1, xq, xk, xv))
                if st >= 1:
                    # wo for qt=st-1: drained by filler inside attn(st)
                    pending.extend(wo_units(st - 1))
                attn(st, filler)
            # drain the tail
            filler(len(pending))
            pending.extend(wo_units(NST - 1))
            filler(len(pending))

    nc.compile()
    return nc


def _prep_inputs(Q, K, V, Wq_w, Wq_b, Wk_w, Wk_b, Wv_w, Wv_b, Wo_w, Wo_b):
    bf = ml_dtypes.bfloat16
    f = np.float32

    def xprep(X):
        # [S, D] -> [128, 8, S]: x[p, t, s] = X[s, t*128+p]
        return np.ascontiguousarray(
            X[0].T.reshape(8, 128, S).transpose(1, 0, 2).astype(bf)
        )

    def wprep(Wslice):
        # Wslice [CD, D] -> [128, 8, CD]: w[p, t, c] = Wslice[c, t*128+p]
        return np.ascontiguousarray(
            Wslice.T.reshape(8, 128, CD).transpose(1, 0, 2).astype(bf)
        )

    XQp, XKp, XVp = xprep(Q), xprep(K), xprep(V)
    p = np.arange(KB)[:, None]
    fidx = np.arange(KB)[None, :]
    msk = np.where(p <= fidx, 1.0, 0.0).astype(bf)
    WoT = np.ascontiguousarray(Wo_w.T, dtype=f)  # [in, out]

    in_maps = []
    for c in range(NCORES):
        c0 = CD * c
        in_maps.append({
            "xq": XQp, "xk": XKp, "xv": XVp,
            "wq": wprep(Wq_w[c0 : c0 + CD, :]),
            "wk": wprep(Wk_w[c0 : c0 + CD, :]),
            "wv": wprep(Wv_w[c0 : c0 + CD, :]),
            "bq": np.ascontiguousarray(Wq_b[c0 : c0 + CD, None], dtype=f),
            "bk": np.ascontiguousarray(Wk_b[c0 : c0 + CD, None], dtype=f),
            "bvb": np.ascontiguousarray(
                np.broadcast_to(np.tile(Wv_b[c0 : c0 + CD], 4), (128, 512))
            ).astype(bf),
            "wor": np.ascontiguousarray(WoT[c0 : c0 + CD, :], dtype=bf),
            "msk": msk,
        })
    return in_maps


def _numpy_fallback(Q, K, V, Wq_w, Wq_b, Wk_w, Wk_b, Wv_w, Wv_b, Wo_w, Wo_b,
                    mask):
    q = (Q @ Wq_w.T + Wq_b).reshape(1, S, H, DK).transpose(0, 2, 1, 3)
    k = (K @ Wk_w.T + Wk_b).reshape(1, S, H, DK).transpose(0, 2, 1, 3)
    v = (V @ Wv_w.T + Wv_b).reshape(1, S, H, DK).transpose(0, 2, 1, 3)
    scores = np.einsum("bhqd,bhkd->bhqk", q, k) / np.sqrt(DK).astype(np.float32)
    scores = np.where(mask == 0, np.float32(-1e9), scores)
    scores -= scores.max(axis=-1, keepdims=True)
    e = np.exp(scores)
    attn = e / e.sum(axis=-1, keepdims=True)
    out = np.einsum("bhqk,bhkd->bhqd", attn, v)
    out = out.transpose(0, 2, 1, 3).reshape(1, S, D)
    return (out @ Wo_w.T + Wo_b).astype(np.float32)


def kernel(Q, K, V, Wq_w, Wq_b, Wk_w, Wk_b, Wv_w, Wv_b, Wo_w, Wo_b, mask,
           **run_kwargs):
    Q = np.asarray(Q); K = np.asarray(K); V = np.asarray(V)
    Wq_w = np.asarray(Wq_w); Wq_b = np.asarray(Wq_b)
    Wk_w = np.asarray(Wk_w); Wk_b = np.asarray(Wk_b)
    Wv_w = np.asarray(Wv_w); Wv_b = np.asarray(Wv_b)
    Wo_w = np.asarray(Wo_w); Wo_b = np.asarray(Wo_b)
    mask = np.asarray(mask)

    causal = np.array_equal(
        mask.reshape(S, S), np.tril(np.ones((S, S), mask.dtype))
    )
    if not causal:
        return _numpy_fallback(Q, K, V, Wq_w, Wq_b, Wk_w, Wk_b, Wv_w, Wv_b,
                               Wo_w, Wo_b, mask)

    from concourse.bass_utils import run_bass_kernel_spmd

    if _compiled[0] is None:
        _compiled[0] = _build()
    nc = _compiled[0]

    in_maps = _prep_inputs(Q, K, V, Wq_w, Wq_b, Wk_w, Wk_b, Wv_w, Wv_b,
                           Wo_w, Wo_b)
    res = run_bass_kernel_spmd(nc, in_maps, list(range(NCORES)), **run_kwargs)
    out = np.zeros((S, D), np.float32)
    for cres in res.results:
        out += np.asarray(cres["out"], dtype=np.float32)
    out += Wo_b.astype(np.float32)
    if run_kwargs:
        kernel.last_result = res
    return out.reshape(1, S, D).astype(np.float32)
